# revision 2
# baseline (speedup 1.0000x reference)
"""Trainium2 Bass kernel for nn_Conv2d_77489799955262.

Forward value of the reference:
    y = conv2d(x, (w_pos > 0) - (w_neg > 0))      # ternary weights in {-1, 0, 1}
(the straight-through-estimator terms cancel numerically), NCHW, 3x3, stride 1,
pad 1, x [32, 256, 56, 56] f32, w [256, 256, 3, 3].

Strategy: data-parallel over batch across 8 cores (4 images per core).
Implicit GEMM with fp8 DoubleRow matmuls (0.5 PE cycles per output column,
2x the bf16/f32r streaming rate): x is split on the host into
    hi = fp8_e4m3(x),  lo = fp8_e4m3(x - hi)
Products of fp8 values with ternary {-1,0,1} weights are exact, so the only
error is the hi+lo representation of x (~2^-9 relative; measured end-to-end
max rel err ~7e-4).

Each PSUM row-block tile accumulates 18 DoubleRow matmuls: 9 taps x {hi,lo},
with the two 128-wide ci blocks PAIRED in the doubled contraction dim
(lhsT [128, 2, 128co], rhs [128, 2, 456]). Rows are processed 8 at a time
over a zero-padded row layout with row stride 57 (the right pad of row r and
the left pad of row r+1 share one zero); the host pre-pads rows to 57 wide
([0, row]) so each (image, ci-block) input DMA is one contiguous 3192B run
per partition. Output drains into a full-image [128, 3136] f32 staging tile
-> one DMA per (image, co-block) with 12.5KB-per-partition descriptors.
"""
import numpy as np
import ml_dtypes

import concourse.bass as bass
import concourse.tile as tile
from concourse import bacc, mybir
from concourse.bass_utils import run_bass_kernel_spmd

MODE = "fp8dr"

N_CORES = 8
B, CI, CO, H, W, K = 32, 256, 256, 56, 56, 3
NI = B // N_CORES          # images per core
PH, PW = H + 2, W + 1      # padded rows; row stride is 57
IMG = PH * PW              # 3306 padded elems per image per channel
CIB = CI // 128            # ci blocks (paired in the DoubleRow K dim)
COB = CO // 128            # co blocks
RB = 8                     # output rows per psum tile
NMM = RB * PW              # matmul moving free dim: 456
NRB = H // RB              # 7 row blocks per image
SLACK = 64                 # per-ci-block trailing zeros for edge windows
S = NI * IMG + SLACK       # plane stride per ci block: 13288
XLEN = CIB * S             # 26576

F32 = mybir.dt.float32
FP8 = mybir.dt.float8e4
NPF8 = ml_dtypes.float8_e4m3

_COMPILED = {}


def _build(mode="fp8dr", iters=1, loop=0):
    nc = bacc.Bacc("TRN2", target_bir_lowering=False, debug=False,
                   num_devices=N_CORES)

    xh_dram = nc.dram_tensor("xh", [NI, CI, H, PW], FP8, kind="ExternalInput")
    xl_dram = nc.dram_tensor("xl", [NI, CI, H, PW], FP8, kind="ExternalInput")
    w_dram = nc.dram_tensor("w", [CI, 9, CO], FP8, kind="ExternalInput")
    y_dram = nc.dram_tensor("y", [NI, CO, H, W], F32, kind="ExternalOutput")
    xdrams = (xh_dram, xl_dram)

    with tile.TileContext(nc) as tc:
        with (
            tc.tile_pool(name="const", bufs=1) as cpool,
            tc.tile_pool(name="outp", bufs=3) as opool,
            tc.tile_pool(name="psum", bufs=8, space="PSUM") as ppool,
        ):
            w_sb = cpool.tile([128, CIB, 9, CO], FP8, tag="w")
            nc.sync.dma_start(w_sb[:, 0], w_dram[0:128])
            nc.sync.dma_start(w_sb[:, 1], w_dram[128:256])

            planes = [cpool.tile([128, XLEN], FP8, tag=f"xp{p}",
                                 name=f"xp{p}") for p in range(2)]
            # [128, ci_block, S] views; the DoubleRow pair dim strides by S
            pviews = [pl[:].rearrange("p (c s) -> p c s", c=CIB)
                      for pl in planes]

            def off(ci, n):
                return ci * S + n * IMG

            def emit_iter(it):
                # pad rows 0 and 57 per (plane, ci, n); slack once
                for p in range(2):
                    for ci in range(CIB):
                        for n in range(NI):
                            eng = nc.vector if n == 0 else nc.gpsimd
                            o = off(ci, n)
                            eng.memset(planes[p][:, o:o + PW], 0.0)
                            eng.memset(
                                planes[p][:, o + (PH - 1) * PW:o + IMG], 0.0)
                        if it == 0:
                            nc.gpsimd.memset(
                                planes[p][:, ci * S + NI * IMG:(ci + 1) * S],
                                0.0)

                # input DMAs: rows 1..56 of each plane are one contiguous
                # 3192B-per-partition run (host pre-pads rows to 57 wide)
                for n in range(NI):
                    for ci in range(CIB):
                        for p in range(2):
                            o = off(ci, n)
                            dst = (planes[p][:, o + PW:o + PW + H * PW]
                                   .rearrange("p (r c) -> p r c", c=PW))
                            src = xdrams[p][n, ci * 128:(ci + 1) * 128, :, :]
                            q = nc.sync if n == 0 else nc.gpsimd
                            q.dma_start(dst, src)

                # main matmul loops: one group of 7 row-block psum tiles per
                # (image, co block); each accumulates 9 taps x {hi, lo}
                for n in range(NI):
                    for co in range(COB):
                        ot = opool.tile([128, H * W], F32, tag="ot",
                                        name=f"ot_{it}_{n}_{co}")
                        ot_v = ot[:].rearrange("p (r c) -> p r c", c=W)
                        pss = [ppool.tile([128, NMM], F32, tag="ps",
                                          name=f"ps_{it}_{n}_{co}_{r}")
                               for r in range(NRB)]
                        for tap in range(9):
                            kh, kw = divmod(tap, 3)
                            lhsT = w_sb[:, :, tap, co * 128:(co + 1) * 128]
                            for p in range(2):
                                for r in range(NRB):
                                    base = n * IMG + (r * RB + kh) * PW + kw
                                    nc.tensor.matmul(
                                        pss[r][:], lhsT,
                                        pviews[p][:, :, base:base + NMM],
                                        start=(tap == 0 and p == 0),
                                        stop=(tap == 8 and p == 1),
                                        perf_mode=mybir.MatmulPerfMode
                                        .DoubleRow)
                        for r in range(NRB):
                            src = (pss[r][:]
                                   .rearrange("p (i j) -> p i j", j=PW)
                                   [:, :, 0:W])
                            nc.vector.tensor_copy(
                                ot_v[:, r * RB:(r + 1) * RB, :], src)
                        nc.sync.dma_start(
                            y_dram[n, co * 128:(co + 1) * 128, :, :], ot[:])

            if loop:
                with tc.For_i(0, loop, 1,
                              hint_engines=(mybir.EngineType.PE,)):
                    emit_iter(0)
            else:
                for it in range(iters):
                    emit_iter(it)

    nc.compile()
    return nc


def _get_compiled(mode):
    if mode not in _COMPILED:
        _COMPILED[mode] = _build(mode)
    return _COMPILED[mode]


def _prep_inputs(x, w_pos, w_neg):
    """Host-side prep: ternary weights -> fp8 lhsT; x -> padded fp8 hi/lo."""
    w_eff = ((w_pos > 0).astype(np.float32)
             - (w_neg > 0).astype(np.float32))          # [CO, CI, 3, 3]
    w_lhsT = np.ascontiguousarray(
        w_eff.reshape(CO, CI, 9).transpose(1, 2, 0)).astype(NPF8)

    x = np.ascontiguousarray(x, dtype=np.float32)
    hi = x.astype(NPF8)
    lo = (x - hi.astype(np.float32)).astype(NPF8)
    xh = np.zeros((B, CI, H, PW), NPF8)
    xl = np.zeros((B, CI, H, PW), NPF8)
    xh[..., 1:] = hi
    xl[..., 1:] = lo

    return [
        {"xh": np.ascontiguousarray(xh[c * NI:(c + 1) * NI]),
         "xl": np.ascontiguousarray(xl[c * NI:(c + 1) * NI]),
         "w": w_lhsT}
        for c in range(N_CORES)
    ]


def kernel(x, w_pos, w_neg):
    nc = _get_compiled(MODE)
    in_maps = _prep_inputs(x, w_pos, w_neg)
    res = run_bass_kernel_spmd(nc, in_maps, list(range(N_CORES)))
    out = np.concatenate([res.results[c]["y"] for c in range(N_CORES)], axis=0)
    return out.astype(np.float32)
